# revision 27
# baseline (speedup 1.0000x reference)
"""GCN (3-layer, PyG-style symmetric norm) on 8 Trainium2 NeuronCores.

Strategy (hardcoded for N=50000, E=800000, C=128, 8 cores):
  - Nodes sharded by contiguous ranges of 6250 across 8 cores; edges
    partitioned by dst so segment-sum is local to the dst owner.
  - Per layer: every core computes the scaled feature table slab
    t = dinv * (h @ W) for its own nodes, an AllGather makes the full
    [50000, 128] table visible to all cores, then each core gathers the
    rows for its edges' sources with dma_gather (512B rows) and
    segment-sums them into per-dst-block accumulators with one-hot
    matmuls on the TensorEngine (transposed accumulation: PSUM holds
    aggT[ch, node]).
  - dma_gather indices are int16, so the table is addressed as two
    halves (rows < 32768 and >= 32768) and each dst block's edges are
    split into low/high chunks of 128.
  - Final layer: column-reduce dinv-scaled aggregates; host adds bias
    b3 and divides by N.
"""

import sys

for _p in ("/opt/trn_rl_repo", "/root/.axon_site/_ro/trn_rl_repo"):
    if _p not in sys.path:
        sys.path.insert(0, _p)

import numpy as np


class GCNConfig:
    """Node ownership: core r owns lo-range [r*LO_PER, (r+1)*LO_PER) and
    hi-range [SPLIT + r*HI_PER, SPLIT + (r+1)*HI_PER).  SPLIT = M*LO_PER
    keeps both gather tables int16-addressable and offset-free."""

    def __init__(self, n_nodes=50000, n_edges=800000, n_cores=8,
                 lo_per_core=4096, blocks_per_group=4, gather_call_chunks=15):
        assert n_nodes % n_cores == 0
        self.N = n_nodes
        self.E = n_edges
        self.C = 128
        self.M = n_cores
        self.LN = n_nodes // n_cores          # local nodes per core
        self.LO_PER = min(lo_per_core, self.LN)
        self.SPLIT = self.LO_PER * n_cores
        self.HI_PER = self.LN - self.LO_PER
        assert self.LO_PER <= 32768 and self.HI_PER <= 32767
        self.NBLK_LO = -(-self.LO_PER // 128)
        self.NBLK_HI = -(-self.HI_PER // 128) if self.HI_PER else 0
        self.NBLK = self.NBLK_LO + self.NBLK_HI
        self.NPAD = self.NBLK * 128           # padded local node count
        self.GB = blocks_per_group            # blocks per gather group
        self.GCC = gather_call_chunks         # max chunks per dma_gather call
        assert self.HI_PER == 0 or self.LO_PER % 128 == 0

    def storage_range(self, r):
        """Storage positions of core r's local ordering [0, LN)."""
        g = np.empty(self.LN, dtype=np.int64)
        g[:self.LO_PER] = r * self.LO_PER + np.arange(self.LO_PER)
        if self.HI_PER:
            g[self.LO_PER:] = (self.SPLIT + r * self.HI_PER
                               + np.arange(self.HI_PER))
        return g


def _balance_positions(cfg, w_node):
    """Assign nodes to storage positions, balancing per-(core, block) edge
    loads within each half. Returns pos[node] -> storage position."""
    import heapq
    N, M, SPLIT = cfg.N, cfg.M, cfg.SPLIT
    pos = np.empty(N, dtype=np.int64)
    for half in (0, 1):
        if half == 0:
            ids = np.arange(0, SPLIT)
            nblk, per = cfg.NBLK_LO, cfg.LO_PER
            base = 0
        else:
            if cfg.HI_PER == 0:
                break
            ids = np.arange(SPLIT, N)
            nblk, per = cfg.NBLK_HI, cfg.HI_PER
            base = SPLIT
        # bins: (core, block) with capacity = blk width
        bins = []
        cap = {}
        fill = {}
        for r in range(M):
            for b in range(nblk):
                w = min(128, per - b * 128)
                bins.append((0.0, (r, b)))
                cap[(r, b)] = w
                fill[(r, b)] = []
        heapq.heapify(bins)
        order = ids[np.argsort(-w_node[ids], kind="stable")]
        spill = []
        for n in order:
            while True:
                load, key = heapq.heappop(bins)
                if len(fill[key]) < cap[key]:
                    break
                spill_dummy = None  # full bin: drop it permanently
            fill[key].append(n)
            if len(fill[key]) < cap[key]:
                heapq.heappush(bins, (load + float(w_node[n]), key))
        for (r, b), members in fill.items():
            start = base + r * per + b * 128
            for i, n in enumerate(members):
                pos[n] = start + i
    return pos


def host_prep(cfg, x, edge_index, dinv=None):
    """Build per-core input arrays + the shared chunk schedule.

    Returns (schedule_meta, per_core_inputs:list[dict]).
    """
    N, M, LN, NBLK, SPLIT = cfg.N, cfg.M, cfg.LN, cfg.NBLK, cfg.SPLIT

    src0 = np.asarray(edge_index[0], dtype=np.int64)
    dst0 = np.asarray(edge_index[1], dtype=np.int64)

    if dinv is None:
        # degree includes the self-loop (+1); self-loops are applied on
        # device via an identity matmul, not gathered.
        deg = (np.bincount(dst0, minlength=N) + 1).astype(np.float32)
        dinv = (1.0 / np.sqrt(deg)).astype(np.float32)

    # balance (core, block) bin loads; nodes keep their half
    w_node = np.bincount(dst0, minlength=N).astype(np.float64)
    pos = _balance_positions(cfg, w_node)
    inv = np.empty(N, dtype=np.int64)
    inv[pos] = np.arange(N)

    src_all = pos[src0]
    dst_all = pos[dst0]

    # dst position -> (owner core, local index) under the lo/hi ownership
    is_hi_dst = dst_all >= SPLIT
    q = dst_all - SPLIT
    core = np.where(is_hi_dst, q // max(cfg.HI_PER, 1), dst_all // cfg.LO_PER)
    li = np.where(is_hi_dst, cfg.LO_PER + q % max(cfg.HI_PER, 1),
                  dst_all % cfg.LO_PER)
    blk = li // 128
    dloc = li % 128
    half = (src_all >= SPLIT).astype(np.int64)

    key = (core * NBLK + blk) * 2 + half
    order = np.argsort(key, kind="stable")
    s_src = src_all[order]
    s_dloc = dloc[order]
    counts = np.bincount(key, minlength=M * NBLK * 2).reshape(M, NBLK, 2)
    starts = np.zeros(M * NBLK * 2 + 1, dtype=np.int64)
    np.cumsum(counts.reshape(-1), out=starts[1:])

    # shared per-(block, half) slot counts: max over cores
    R = counts.max(axis=0)  # [NBLK, 2]

    groups = [list(range(g, min(g + cfg.GB, NBLK)))
              for g in range(0, NBLK, cfg.GB)]
    # schedule: per group: lo chunks then hi chunks.  Full 128-slot runs of
    # a block get single-piece chunks; trailing partial runs of the group's
    # blocks are packed into shared "mixed" chunks (one matmul piece per
    # block slot-range, masked via dstT = -1 outside the range).
    schedule = []      # per chunk: list of (block, piece_col, slot0, nslots)
    gh_ranges = {}     # (gi, half) -> (chunk0, nchunks)
    npiece = 0
    for gi, grp in enumerate(groups):
        for h in (0, 1):
            c0 = len(schedule)
            # full chunks
            for b in grp:
                for _ in range(int(R[b, h]) // 128):
                    schedule.append([(b, npiece, 0, 128)])
                    npiece += 1
            # packed trailing partials
            cur = None
            used = 0
            for b in grp:
                rem = int(R[b, h]) % 128
                while rem > 0:
                    if cur is None:
                        cur = []
                        schedule.append(cur)
                        used = 0
                    take = min(rem, 128 - used)
                    cur.append((b, npiece, used, take))
                    npiece += 1
                    used += take
                    rem -= take
                    if used == 128:
                        cur = None
            gh_ranges[(gi, h)] = (c0, len(schedule) - c0)
    NCHUNK = len(schedule)
    NPIECE = npiece

    # per-chunk slot ranges in per-(core,block,half) order
    per_core = []
    for r in range(M):
        idx_arr = np.zeros((NCHUNK, 128), dtype=np.int64)
        dl_arr = np.full((NPIECE, 128), -1.0, dtype=np.float32)
        # walk schedule, fill slot ranges from this core's sorted runs
        pos_in = {}
        for gi, grp in enumerate(groups):
            for h in (0, 1):
                c0, nck = gh_ranges[(gi, h)]
                for ci in range(c0, c0 + nck):
                    for (b, pc, s0, ns) in schedule[ci]:
                        k = pos_in.get((b, h), 0)
                        pos_in[(b, h)] = k + ns
                        kk = (r * NBLK + b) * 2 + h
                        lo, hi = starts[kk], starts[kk + 1]
                        a = lo + k
                        nreal = max(0, min(ns, hi - a))
                        if nreal > 0:
                            seg = slice(a, a + nreal)
                            sv = s_src[seg]
                            idx_arr[ci, s0:s0 + nreal] = \
                                sv - (SPLIT if h else 0)
                            dl_arr[pc, s0:s0 + nreal] = s_dloc[seg]
        # wrapped int16 layout: idx i -> [i%16 (+16k), i//16]
        flat = idx_arr.reshape(-1)
        w16 = flat.reshape(-1, 16).T.astype(np.int16)  # [16, NCHUNK*8]
        idxw = np.tile(w16, (8, 1))                    # [128, NCHUNK*8]
        dstT = np.ascontiguousarray(dl_arr.T)          # [128, NPIECE]

        g = inv[cfg.storage_range(r)]
        xs = np.asarray(x[g], dtype=np.float32)
        xT = np.zeros((128, cfg.NPAD), dtype=np.float32)
        xT[:, :LN] = xs.T
        dv = np.zeros(cfg.NPAD, dtype=np.float32)
        dv[:LN] = dinv[g]
        Dall = np.ascontiguousarray(
            np.broadcast_to(dv[None, :], (128, cfg.NPAD)))
        dinvc = np.ascontiguousarray(dv.reshape(NBLK, 128).T)  # [128, NBLK]

        per_core.append(dict(xT=xT, idxw=idxw, dstT=dstT, Dall=Dall,
                             dinvc=dinvc))

    meta = dict(groups=groups, schedule=schedule, gh_ranges=gh_ranges,
                NCHUNK=NCHUNK, NPIECE=NPIECE)
    return meta, per_core, dinv


def build_program(cfg, meta, debug_layers=(1, 2, 3), debug_no_agg=False,
                  debug_no_epilogue=False, debug_no_matmul=False):
    import concourse.bass as bass
    import concourse.bacc as bacc
    import concourse.tile as tile
    from concourse import mybir
    from contextlib import ExitStack

    f32 = mybir.dt.float32
    f16 = mybir.dt.float16
    i16 = mybir.dt.int16
    N, C, M, LN = cfg.N, cfg.C, cfg.M, cfg.LN
    NBLK, NPAD, SPLIT = cfg.NBLK, cfg.NPAD, cfg.SPLIT

    def blk_width(b):
        if b < cfg.NBLK_LO:
            return min(128, cfg.LO_PER - b * 128)
        return min(128, cfg.HI_PER - (b - cfg.NBLK_LO) * 128)

    def blk_slab_row(b):
        """(which_slab, start_row) for block b's slab write."""
        if b < cfg.NBLK_LO:
            return 0, b * 128
        return 1, (b - cfg.NBLK_LO) * 128
    groups, schedule, NCHUNK = meta["groups"], meta["schedule"], meta["NCHUNK"]
    gh_ranges, NPIECE = meta["gh_ranges"], meta["NPIECE"]

    # per-block matmul-piece counts (accumulation chain lengths)
    total_pieces = [0] * NBLK
    for pieces in schedule:
        for (b, _pc, _s0, _ns) in pieces:
            total_pieces[b] += 1

    nc = bacc.Bacc(None, target_bir_lowering=False, debug=False)
    xT_e = nc.declare_dram_parameter("xT", [128, NPAD], f32, isOutput=False)
    idx_e = nc.declare_dram_parameter("idxw", [128, NCHUNK * 8], i16,
                                      isOutput=False)
    dstT_e = nc.declare_dram_parameter("dstT", [128, NPIECE], f32,
                                       isOutput=False)
    Dall_e = nc.declare_dram_parameter("Dall", [128, NPAD], f32,
                                       isOutput=False)
    dinvc_e = nc.declare_dram_parameter("dinvc", [128, NBLK], f32,
                                        isOutput=False)
    W_e = [nc.declare_dram_parameter(f"W{i}", [C, C], f32, isOutput=False)
           for i in (1, 2, 3)]
    b_e = [nc.declare_dram_parameter(f"b{i}", [C, 1], f32, isOutput=False)
           for i in (1, 2)]
    iota_e = nc.declare_dram_parameter("iota", [128, 128], f32,
                                       isOutput=False)
    ident_e = nc.declare_dram_parameter("ident", [128, 128], f16,
                                        isOutput=False)
    out_e = nc.declare_dram_parameter("out_parts", [128, NBLK], f32,
                                      isOutput=True)

    with TileCtx(nc, tile) as tc, ExitStack() as ctx:
        const = ctx.enter_context(tc.tile_pool(name="const", bufs=1))
        dram = ctx.enter_context(tc.tile_pool(name="dram", bufs=1,
                                              space="DRAM"))
        gpool = ctx.enter_context(tc.tile_pool(name="gath", bufs=2))
        epool = ctx.enter_context(tc.tile_pool(name="e2n", bufs=4))
        hpool = ctx.enter_context(tc.tile_pool(name="hsb", bufs=3))
        tpool = ctx.enter_context(tc.tile_pool(name="tsb", bufs=3))
        mpool = ctx.enter_context(tc.tile_pool(name="tmp", bufs=3))
        psA = ctx.enter_context(tc.tile_pool(name="psA", bufs=6,
                                             space="PSUM"))
        psZ = ctx.enter_context(tc.tile_pool(name="psZ", bufs=2,
                                             space="PSUM"))

        xT_sb = const.tile([128, NPAD], f32)
        nc.sync.dma_start(out=xT_sb[:], in_=xT_e[:])
        idx_sb = const.tile([128, NCHUNK * 8], i16)
        nc.sync.dma_start(out=idx_sb[:], in_=idx_e[:])
        dstT_sb = const.tile([128, NPIECE], f32)
        nc.sync.dma_start(out=dstT_sb[:], in_=dstT_e[:])
        Dall_sb = const.tile([128, NPAD], f32)
        nc.sync.dma_start(out=Dall_sb[:], in_=Dall_e[:])
        dinvc_sb = const.tile([128, NBLK], f32)
        nc.sync.dma_start(out=dinvc_sb[:], in_=dinvc_e[:])
        iota_sb = const.tile([128, 128], f32)
        nc.sync.dma_start(out=iota_sb[:], in_=iota_e[:])
        ident_sb = const.tile([128, 128], f16)
        nc.sync.dma_start(out=ident_sb[:], in_=ident_e[:])
        W_sb = []
        for i in range(3):
            w = const.tile([128, 128], f32, name=f"w{i}_sb")
            nc.sync.dma_start(out=w[:], in_=W_e[i][:])
            W_sb.append(w)
        bias_sb = []
        for i in range(2):
            bcol = const.tile([128, 1], f32, name=f"b{i}_sb")
            nc.sync.dma_start(out=bcol[:], in_=b_e[i][:])
            bias_sb.append(bcol)
        parts_sb = const.tile([128, NBLK], f32)
        nc.vector.memset(parts_sb[:], 0.0)

        # NOTE: gather tables are Local (not Shared) and exactly sized with
        # zero AP offset — dma_gather's Q7 descriptor generator crashes the
        # device on Shared-scratchpad or offset-view sources.
        slab_lo = dram.tile([cfg.LO_PER, C], f16)
        t_lo_fulls = [dram.tile([SPLIT, C], f16, name=f"t_lo_l{i}")
                      for i in (1, 2, 3)]
        if cfg.HI_PER:
            slab_hi = dram.tile([cfg.HI_PER, C], f16)
            t_hi_fulls = [dram.tile([N - SPLIT, C], f16, name=f"t_hi_l{i}")
                          for i in (1, 2, 3)]

        rg = [list(range(M))]

        # dummy gather to prefetch the Q7 dma_gather library: the iram load
        # (~60us) otherwise stalls the first real gather of layer 1
        warm_sb = const.tile([128, 1, 128], f16, name="warm_sb")
        nc.gpsimd.dma_gather(
            warm_sb[:], t_lo_fulls[0][:], idx_sb[:, 0:8],
            num_idxs=128, num_idxs_reg=128, elem_size=C,
            single_packet=False)

        def slab_block(b, lhsT_ap, w_sb):
            z_ps = psZ.tile([128, 128], f32, tag="zps")
            nc.tensor.matmul(out=z_ps[:], lhsT=lhsT_ap, rhs=w_sb[:],
                             start=True, stop=True)
            t_sb = tpool.tile([128, 128], f16, tag="tsb")
            nc.scalar.activation(out=t_sb[:], in_=z_ps[:],
                                 func=mybir.ActivationFunctionType.Copy,
                                 scale=dinvc_sb[:, b:b + 1])
            w = blk_width(b)
            which, row = blk_slab_row(b)
            slab = slab_lo if which == 0 else slab_hi
            nc.sync.dma_start(out=slab[row:row + w, :], in_=t_sb[:w, :])

        # layer-1 table slab from the input features
        for b in range(NBLK):
            slab_block(b, xT_sb[:, b * 128:(b + 1) * 128], W_sb[0])

        for layer in debug_layers:
            nc.gpsimd.collective_compute(
                "AllGather", mybir.AluOpType.bypass, replica_groups=rg,
                ins=[slab_lo[:]], outs=[t_lo_fulls[layer - 1][:]])
            if cfg.HI_PER:
                nc.gpsimd.collective_compute(
                    "AllGather", mybir.AluOpType.bypass, replica_groups=rg,
                    ins=[slab_hi[:]], outs=[t_hi_fulls[layer - 1][:]])
            if debug_no_agg:
                continue

            agg_tiles = {}
            chain_pos = [0] * NBLK
            for gi, grp in enumerate(groups):
                if not (debug_no_matmul or debug_no_epilogue):
                    for b in grp:
                        agg_tiles[b] = psA.tile(
                            [128, 128], f32, tag="agg",
                            name=f"agg_l{layer}_b{b}")[:]
                    # self-loop contribution: aggT_b starts as t_blk.T via an
                    # identity matmul on this core's own slab rows
                    for b in grp:
                        w = blk_width(b)
                        which, row = blk_slab_row(b)
                        slab = slab_lo if which == 0 else slab_hi
                        trow = tpool.tile([128, 128], f16, tag="trow",
                                          name=f"trow_l{layer}_b{b}")
                        nc.sync.dma_start(out=trow[:w, :],
                                          in_=slab[row:row + w, :])
                        nc.tensor.matmul(
                            out=agg_tiles[b], lhsT=trow[:w, :],
                            rhs=ident_sb[:w, :],
                            start=True, stop=(total_pieces[b] == 0))
                for half in (0, 1):
                    c0, nck = gh_ranges[(gi, half)]
                    if nck == 0:
                        continue
                    gt = gpool.tile([128, nck, 128], f16, tag=f"g{half}")
                    src_view = (t_lo_fulls[layer - 1][:] if half == 0
                                else t_hi_fulls[layer - 1][:])
                    # single_packet=False: single-packet mode fails on HW
                    # above 1024 indices per call.  Calls are capped at GCC
                    # chunks; hard limit ~125 (dma_gather's Q7 kernel stages
                    # num_idxs int32s in the 64KB scratch buffer).
                    for s0 in range(0, nck, cfg.GCC):
                        s1 = min(s0 + cfg.GCC, nck)
                        nc.gpsimd.dma_gather(
                            gt[:, s0:s1, :], src_view,
                            idx_sb[:, (c0 + s0) * 8:(c0 + s1) * 8],
                            num_idxs=(s1 - s0) * 128,
                            num_idxs_reg=(s1 - s0) * 128,
                            elem_size=C, single_packet=False)
                    if debug_no_matmul:
                        continue
                    for j in range(nck):
                        for (b, pc, _s0, _ns) in schedule[c0 + j]:
                            e2 = epool.tile([128, 128], f16, tag="e2n")
                            nc.vector.tensor_scalar(
                                out=e2[:], in0=iota_sb[:],
                                scalar1=dstT_sb[:, pc:pc + 1], scalar2=None,
                                op0=mybir.AluOpType.is_equal)
                            nc.tensor.matmul(
                                out=agg_tiles[b],
                                lhsT=gt[:, j, :], rhs=e2[:],
                                start=False,
                                stop=(chain_pos[b] == total_pieces[b] - 1))
                            chain_pos[b] += 1
                if debug_no_epilogue or debug_no_matmul:
                    continue
                # epilogue for the blocks of this group
                for b in grp:
                    tmp = mpool.tile([128, 128], f32, tag="tmp")
                    nc.vector.tensor_tensor(
                        out=tmp[:], in0=agg_tiles[b],
                        in1=Dall_sb[:, b * 128:(b + 1) * 128],
                        op=mybir.AluOpType.mult)
                    if layer < 3:
                        h_sb = hpool.tile([128, 128], f32, tag="hsb")
                        nc.scalar.activation(
                            out=h_sb[:], in_=tmp[:],
                            func=mybir.ActivationFunctionType.Relu,
                            bias=bias_sb[layer - 1][:])
                        slab_block(b, h_sb[:], W_sb[layer])
                    else:
                        nc.vector.reduce_sum(
                            out=parts_sb[:, b:b + 1], in_=tmp[:],
                            axis=mybir.AxisListType.X)
        nc.sync.dma_start(out=out_e[:], in_=parts_sb[:])

    nc.compile()
    return nc


# small helper so build_program can use `with TileCtx(...)`
def TileCtx(nc, tile_mod):
    return tile_mod.TileContext(nc)


def run(cfg, meta, per_core, weights, trace=False):
    from concourse.bass_utils import run_bass_kernel_spmd

    nc = build_program(cfg, meta)
    iota = np.ascontiguousarray(
        np.broadcast_to(np.arange(128, dtype=np.float32)[None, :],
                        (128, 128)))
    ident = np.eye(128, dtype=np.float16)
    in_maps = []
    for r in range(cfg.M):
        m = dict(per_core[r])
        m["W1"], m["W2"], m["W3"] = weights["W1"], weights["W2"], weights["W3"]
        m["b1"] = weights["b1"].reshape(cfg.C, 1).astype(np.float32)
        m["b2"] = weights["b2"].reshape(cfg.C, 1).astype(np.float32)
        m["iota"] = iota
        m["ident"] = ident
        in_maps.append(m)
    res = run_bass_kernel_spmd(nc, in_maps, core_ids=list(range(cfg.M)),
                               trace=trace)
    return res


def kernel(**inputs):
    cfg = GCNConfig()
    x = np.asarray(inputs["x"], dtype=np.float32)
    meta, per_core, dinv = host_prep(cfg, x, inputs["edge_index"])
    weights = {k: np.asarray(inputs[k], dtype=np.float32)
               for k in ("W1", "b1", "W2", "b2", "W3", "b3")}
    res = run(cfg, meta, per_core, weights, trace=False)
    total = np.zeros(cfg.C, dtype=np.float64)
    for r in range(cfg.M):
        total += res.results[r]["out_parts"].astype(np.float64).sum(axis=1)
    out = total / cfg.N + weights["b3"].astype(np.float64)
    return out.astype(np.float32)

